# revision 1
# baseline (speedup 1.0000x reference)
"""Trainium2 Bass kernel for nn_ChebNet (complex Chebyshev GNN layer).

Sharding: data-parallel over batch B=8 across the 8 NeuronCores (one batch
element per core). No inter-core communication.

v2 design (vs. 373us baseline): transposed attention layout + PE-diag
products + bf16.

Per-core pipeline (batch b, N=1024 nodes, C=OC=64, K+1=5):
  A) XT build (PE transposes), fp32 attention projections, src rows
     broadcast over partitions (attention tiles are TRANSPOSED: [j, i]),
     dst rows transposed to per-chunk bias columns. A_k = Xr@w_r[k] and
     B_k = Xi@w_i[k] stacks precomputed (bf16).
  B) per j-chunk [128, 1024]: s = Prelu(src_bc + dst_col) on ACT (native
     parametric relu), fp32 mag chain using only {square, ln, exp} (one
     act table): m2=sr2+si2, lnm2=Ln(m2+eps), mag=Exp(0.5*lnm2),
     e=Exp(mag) with fused accum_out -> softmax colsum (free-dim sum = the
     softmax axis in transposed layout), er=e/mag=Exp(mag-0.5*lnm2).
     a tiles in ONE fp32 stt with per-partition 1/colsum scalar:
     ar = (sTr * invc) * er -> bf16. mai = -ai.
  C) products SLr[k,i]=sum_j Lr^T*ar - Li^T*ai via PE diag trick: bf16
     matmuls arT-block x LT-block accumulate [i',i] PSUM tiles over all
     chunks; diag extracted in batches (block-diag mask stt + one
     tensor_reduce). L is host-transposed+cast to bf16 (halves HBM).
     k rounds of 2+2+1 to fit PSUM (one bank per 2 i-blocks).
  D) output einsum: out_r = sum_k SLr_k*A_k - SLi_k*B_k via DVE
     tensor_scalar (per-partition SL scalar, 4x mode) into a stacked tile
     + one strided tensor_reduce per output.
"""

import numpy as np
from contextlib import ExitStack

B, N, C, OC = 8, 1024, 64, 64
K1 = 5          # K+1 Chebyshev planes
P = 128         # partitions
CH = N // P     # 8 chunks
NCORES = 8
EPS = 1e-12
ROUNDS = ((0, 1), (2, 3), (4,))

_CACHE = {}


def _build_nc():
    import concourse.tile as tile
    from concourse import bacc, mybir

    f32 = mybir.dt.float32
    bf16 = mybir.dt.bfloat16
    Alu = mybir.AluOpType
    Act = mybir.ActivationFunctionType

    nc = bacc.Bacc("TRN2", target_bir_lowering=False, debug=False,
                   enable_asserts=False, num_devices=NCORES)

    x_real = nc.dram_tensor("x_real", [N, C], f32, kind="ExternalInput").ap()
    x_imag = nc.dram_tensor("x_imag", [N, C], f32, kind="ExternalInput").ap()
    l_realT = nc.dram_tensor("l_realT", [K1, N, N], bf16, kind="ExternalInput").ap()
    l_imagT = nc.dram_tensor("l_imagT", [K1, N, N], bf16, kind="ExternalInput").ap()
    w4 = nc.dram_tensor("w4", [2 * C, 4], f32, kind="ExternalInput").ap()
    wcat = nc.dram_tensor("wcat", [2 * C, K1 * OC], bf16, kind="ExternalInput").ap()
    pa_cols = nc.dram_tensor("pa_cols", [P, 2], f32, kind="ExternalInput").ap()
    ab2 = nc.dram_tensor("ab2", [1, 2], f32, kind="ExternalInput").ap()
    out_r = nc.dram_tensor("out_r", [N, OC], f32, kind="ExternalOutput").ap()
    out_i = nc.dram_tensor("out_i", [N, OC], f32, kind="ExternalOutput").ap()

    with tile.TileContext(nc) as tc:
        with ExitStack() as ctx:
            _emit(ctx, tc, nc, mybir, f32, bf16, Alu, Act,
                  x_real, x_imag, l_realT, l_imagT, w4, wcat,
                  pa_cols, ab2, out_r, out_i)
    nc.compile()
    return nc


def _emit(ctx, tc, nc, mybir, f32, bf16, Alu, Act,
          x_real, x_imag, l_realT, l_imagT, w4, wcat,
          pa_cols, ab2, out_r, out_i):
    X = mybir.AxisListType.X

    # ---------------- constants ----------------
    const = ctx.enter_context(tc.tile_pool(name="const", bufs=1))
    persist = ctx.enter_context(tc.tile_pool(name="persist", bufs=1))

    ident_i = const.tile([P, P], mybir.dt.int32)
    nc.gpsimd.iota(ident_i[:], pattern=[[1, P]], base=0, channel_multiplier=-1)
    ident = const.tile([P, P], f32)
    nc.vector.tensor_scalar(ident[:], ident_i[:], 0, None, op0=Alu.is_equal)

    # block-diagonal mask for diag extraction: mask[p, g, f] = (f == p)
    mask_i = const.tile([P, 16, P], mybir.dt.int32)
    nc.gpsimd.iota(mask_i[:], pattern=[[0, 16], [1, P]], base=0,
                   channel_multiplier=-1)
    mask_bf = const.tile([P, 16, P], bf16)
    nc.vector.tensor_scalar(mask_bf[:], mask_i[:], 0, None, op0=Alu.is_equal)

    ones_row = const.tile([1, P], f32)
    nc.vector.memset(ones_row[:], 1.0)
    ones_row_bf = const.tile([1, P], bf16)
    nc.vector.memset(ones_row_bf[:], 1.0)
    zeros_row_bf = const.tile([1, 512], bf16)
    nc.vector.memset(zeros_row_bf[:], 0.0)
    eps_col = const.tile([P, 1], f32)
    nc.vector.memset(eps_col[:], EPS)

    w4_sb = const.tile([P, 4], f32)
    nc.sync.dma_start(w4_sb[:], w4[:])
    wcat_sb = const.tile([P, K1 * OC], bf16)
    nc.sync.dma_start(wcat_sb[:], wcat[:])
    pa_sb = const.tile([P, 2], f32)
    nc.sync.dma_start(pa_sb[:], pa_cols[:])
    ab_sb = const.tile([1, 2], f32)
    nc.sync.dma_start(ab_sb[:], ab2[:])

    # ---------------- persistent tiles ----------------
    xt = persist.tile([P, N], f32)        # [Xr^T; Xi^T]
    xtb = persist.tile([P, N], bf16)
    src_bc_r = persist.tile([P, N], f32)  # src row broadcast over partitions
    src_bc_i = persist.tile([P, N], f32)
    dstT = persist.tile([P, 2 * CH], f32)  # per chunk: dst_r col, dst_i col

    ap_pool = ctx.enter_context(tc.tile_pool(name="apool", bufs=3 * CH))
    ar_t, ai_t, mai_t = [], [], []
    for c in range(CH):
        ar_t.append(ap_pool.tile([P, N], bf16, tag="apool", name=f"ar{c}"))
        ai_t.append(ap_pool.tile([P, N], bf16, tag="apool", name=f"ai{c}"))
        mai_t.append(ap_pool.tile([P, N], bf16, tag="apool", name=f"mai{c}"))

    ab_stack_pool = ctx.enter_context(tc.tile_pool(name="abst", bufs=2 * CH))
    As_t, Bs_t = [], []
    for c in range(CH):
        As_t.append(ab_stack_pool.tile([P, K1 * OC], bf16, tag="abst",
                                       name=f"As{c}"))
        Bs_t.append(ab_stack_pool.tile([P, K1 * OC], bf16, tag="abst",
                                       name=f"Bs{c}"))

    # SL result tiles per round: [P, CH * nk] (col = ib * nk + kk)
    slR_t, slI_t = [], []
    for r, ks in enumerate(ROUNDS):
        slR_t.append(persist.tile([P, CH * len(ks)], f32, name=f"slR{r}"))
        slI_t.append(persist.tile([P, CH * len(ks)], f32, name=f"slI{r}"))

    or_sb = persist.tile([P, CH * OC], f32)
    oi_sb = persist.tile([P, CH * OC], f32)

    stk_pool = ctx.enter_context(tc.tile_pool(name="stkp", bufs=2 * CH))
    stk_r_t = [stk_pool.tile([P, 2 * K1 * OC], bf16, tag="stkp",
                             name=f"stkr{c}") for c in range(CH)]
    stk_i_t = [stk_pool.tile([P, 2 * K1 * OC], bf16, tag="stkp",
                             name=f"stki{c}") for c in range(CH)]

    # ---------------- phase A ----------------
    pa_pool = ctx.enter_context(tc.tile_pool(name="phaseA", bufs=1))
    with tc.tile_pool(name="pa_psum", bufs=2, space="PSUM") as pa_psum, \
         tc.tile_pool(name="pa_proj", bufs=3, space="PSUM") as pa_proj:
        xr_sb = pa_pool.tile([P, CH, C], f32)
        xi_sb = pa_pool.tile([P, CH, C], f32)
        nc.sync.dma_start(xr_sb[:], x_real.rearrange("(c p) d -> p c d", p=P))
        nc.sync.dma_start(xi_sb[:], x_imag.rearrange("(c p) d -> p c d", p=P))

        for c in range(CH):
            tp = pa_psum.tile([C, P], f32, tag="tp")
            nc.tensor.transpose(tp[:], xr_sb[:, c, :], ident[:])
            nc.vector.tensor_copy(xt[0:C, c * P:(c + 1) * P], tp[:])
            tp2 = pa_psum.tile([C, P], f32, tag="tp")
            nc.tensor.transpose(tp2[:], xi_sb[:, c, :], ident[:])
            nc.scalar.copy(xt[C:2 * C, c * P:(c + 1) * P], tp2[:])

        nc.vector.tensor_copy(xtb[:], xt[:])  # fp32 -> bf16

        # projections, per 512-half: src rows as separate [1, 512] tiles
        # (broadcast inputs must start at partition 0), dst as [2, 512]
        srcr_sb = pa_pool.tile([1, N], f32)
        srci_sb = pa_pool.tile([1, N], f32)
        dst_sb = pa_pool.tile([2, N], f32)
        for h in range(2):
            hs = slice(h * 512, (h + 1) * 512)
            srcr_ps = pa_proj.tile([1, 512], f32, tag="proj")
            nc.tensor.matmul(srcr_ps[:], w4_sb[:, 0:1], xt[:, hs],
                             start=True, stop=True)
            nc.scalar.copy(srcr_sb[:, hs], srcr_ps[:])
            srci_ps = pa_proj.tile([1, 512], f32, tag="proj")
            nc.tensor.matmul(srci_ps[:], w4_sb[:, 1:2], xt[:, hs],
                             start=True, stop=True)
            nc.scalar.copy(srci_sb[:, hs], srci_ps[:])
            dst_ps = pa_proj.tile([2, 512], f32, tag="proj")
            nc.tensor.matmul(dst_ps[:], w4_sb[:, 2:4], xt[:, hs],
                             start=True, stop=True)
            nc.scalar.copy(dst_sb[:, hs], dst_ps[:])
        # fold attention bias into src rows
        nc.vector.tensor_scalar(srcr_sb[:], srcr_sb[:], ab_sb[0:1, 0:1],
                                None, op0=Alu.add)
        nc.vector.tensor_scalar(srci_sb[:], srci_sb[:], ab_sb[0:1, 1:2],
                                None, op0=Alu.add)

    with tc.tile_pool(name="pa_bc", bufs=2, space="PSUM") as pa_bc, \
         tc.tile_pool(name="pa_tpd", bufs=2, space="PSUM") as pa_tpd, \
         tc.tile_pool(name="pa_ab", bufs=2, space="PSUM") as pa_ab:
        # src broadcast tiles [P, N] via rank-1 matmuls
        for row_sb, dstt in ((srcr_sb, src_bc_r), (srci_sb, src_bc_i)):
            for h in range(2):
                hs = slice(h * 512, (h + 1) * 512)
                bc = pa_bc.tile([P, 512], f32, tag="srcbc")
                nc.tensor.matmul(bc[:], ones_row[:], row_sb[:, hs],
                                 start=True, stop=True)
                if h == 0:
                    nc.vector.tensor_copy(dstt[:, hs], bc[:])
                else:
                    nc.scalar.copy(dstt[:, hs], bc[:])

        # dstT columns: per-chunk transpose of dst rows -> [P, 2] cols
        for c in range(CH):
            tp3 = pa_tpd.tile([P, 2], f32, tag="tpd")
            nc.tensor.transpose(tp3[:], dst_sb[:, c * P:(c + 1) * P],
                                ident[0:2, 0:2])
            nc.vector.tensor_copy(dstT[:, 2 * c:2 * c + 2], tp3[:])

        # A/B stacks: A_k = Xr @ w_r[k], B_k = Xi @ w_i[k], all k packed
        for c in range(CH):
            cs = slice(c * P, (c + 1) * P)
            psA = pa_ab.tile([P, K1 * OC], f32, tag="psAB")
            nc.tensor.matmul(psA[:], xtb[0:C, cs], wcat_sb[0:C, :],
                             start=True, stop=True)
            nc.scalar.copy(As_t[c][:], psA[:])
            psB = pa_ab.tile([P, K1 * OC], f32, tag="psAB")
            nc.tensor.matmul(psB[:], xtb[C:2 * C, cs], wcat_sb[C:2 * C, :],
                             start=True, stop=True)
            nc.scalar.copy(Bs_t[c][:], psB[:])

    # ---------------- phase B: transposed attention ----------------
    with tc.tile_pool(name="phaseB", bufs=2) as pb:
        for c in range(CH):
            sTr = pb.tile([P, N], f32, tag="sTr")
            nc.scalar.activation(sTr[:], src_bc_r[:], Act.Prelu,
                                 bias=dstT[:, 2 * c:2 * c + 1],
                                 alpha=pa_sb[:, 0:1])
            sTi = pb.tile([P, N], f32, tag="sTi")
            nc.scalar.activation(sTi[:], src_bc_i[:], Act.Prelu,
                                 bias=dstT[:, 2 * c + 1:2 * c + 2],
                                 alpha=pa_sb[:, 1:2])
            sqr = pb.tile([P, N], f32, tag="sqr")
            nc.scalar.activation(sqr[:], sTr[:], Act.Square)
            sqi = pb.tile([P, N], f32, tag="sqi")
            nc.scalar.activation(sqi[:], sTi[:], Act.Square)
            # m2 (in place over sqr)
            nc.vector.tensor_add(sqr[:], sqr[:], sqi[:])
            lnm2 = pb.tile([P, N], f32, tag="lnm2")
            nc.scalar.activation(lnm2[:], sqr[:], Act.Ln, bias=eps_col[:, 0:1])
            mag = pb.tile([P, N], f32, tag="mag")
            nc.scalar.activation(mag[:], lnm2[:], Act.Exp, scale=0.5)
            e_scr = pb.tile([P, N], bf16, tag="escr")
            cs_col = pb.tile([P, 1], f32, tag="cscol")
            nc.scalar.activation(e_scr[:], mag[:], Act.Exp,
                                 accum_out=cs_col[:])
            # d = mag - 0.5*lnm2  (in place over sqi)
            nc.vector.scalar_tensor_tensor(sqi[:], lnm2[:], -0.5, mag[:],
                                           op0=Alu.mult, op1=Alu.add)
            er = pb.tile([P, N], f32, tag="er")
            nc.scalar.activation(er[:], sqi[:], Act.Exp)
            invc = pb.tile([P, 1], f32, tag="invc")
            nc.vector.reciprocal(invc[:], cs_col[:])
            # a tiles: ar = (sTr * invc) * er  -> bf16 (single rounding)
            nc.vector.scalar_tensor_tensor(ar_t[c][:], sTr[:], invc[:, 0:1],
                                           er[:], op0=Alu.mult, op1=Alu.mult)
            nc.vector.scalar_tensor_tensor(ai_t[c][:], sTi[:], invc[:, 0:1],
                                           er[:], op0=Alu.mult, op1=Alu.mult)
            nc.vector.tensor_scalar(mai_t[c][:], ai_t[c][:], -1.0, None,
                                    op0=Alu.mult)

    # ---------------- phase C: PE-diag products ----------------
    with tc.tile_pool(name="lpool", bufs=6) as lp, \
         tc.tile_pool(name="prod_psum", bufs=8, space="PSUM") as pp, \
         tc.tile_pool(name="maskpool", bufs=4) as mp:
        for r, ks in enumerate(ROUNDS):
            nk = len(ks)
            ibs_per_bank = 2 if nk == 2 else 4
            nbank = CH // ibs_per_bank
            psR = [pp.tile([P, ibs_per_bank, nk, P], f32, tag="pp",
                           name=f"psR{r}_{b4}") for b4 in range(nbank)]
            psI = [pp.tile([P, ibs_per_bank, nk, P], f32, tag="pp",
                           name=f"psI{r}_{b4}") for b4 in range(nbank)]
            # psum 'start' resets the whole bank, so zero each bank with one
            # full-bank matmul and accumulate everything else on top.
            for ps in psR + psI:
                nc.tensor.matmul(ps[:], ones_row_bf[:], zeros_row_bf[:],
                                 start=True, stop=False, skip_group_check=True)
            for c in range(CH):
                cs = slice(c * P, (c + 1) * P)
                lr = lp.tile([P, nk, N], bf16, tag="lr")
                li = lp.tile([P, nk, N], bf16, tag="li")
                for kk, k in enumerate(ks):
                    nc.sync.dma_start(lr[:, kk, :], l_realT[k, cs, :])
                    nc.sync.dma_start(li[:, kk, :], l_imagT[k, cs, :])
                last = (c == CH - 1)
                for ib in range(CH):
                    ibs = slice(ib * P, (ib + 1) * P)
                    oR = psR[ib // ibs_per_bank][:, ib % ibs_per_bank, :, :]
                    oI = psI[ib // ibs_per_bank][:, ib % ibs_per_bank, :, :]
                    rhr = lr[:, :, ibs]
                    rhi = li[:, :, ibs]
                    nc.tensor.matmul(oR, ar_t[c][:, ibs], rhr,
                                     start=False, stop=False,
                                     skip_group_check=True)
                    nc.tensor.matmul(oI, ar_t[c][:, ibs], rhi,
                                     start=False, stop=False,
                                     skip_group_check=True)
                    nc.tensor.matmul(oR, mai_t[c][:, ibs], rhi,
                                     start=False, stop=last,
                                     skip_group_check=True)
                    nc.tensor.matmul(oI, ai_t[c][:, ibs], rhr,
                                     start=False, stop=last,
                                     skip_group_check=True)
            # diag extraction: emit ALL bank-freeing mask-mults first so the
            # next round's PE zero-matmuls can grab PSUM banks ASAP; the
            # reduces (which don't hold banks) come after.
            nblk = CH * nk
            maskedR = mp.tile([P, nblk, P], bf16, tag="masked")
            maskedI = mp.tile([P, nblk, P], bf16, tag="masked")
            for ps_list, masked in ((psR, maskedR), (psI, maskedI)):
                for b4 in range(nbank):
                    nblk_b = ibs_per_bank * nk
                    ms = slice(b4 * nblk_b, (b4 + 1) * nblk_b)
                    nc.vector.scalar_tensor_tensor(
                        masked[:, ms, :], ps_list[b4][:], 1.0,
                        mask_bf[:, 0:nblk_b, :], op0=Alu.bypass, op1=Alu.mult)
            nc.vector.reduce_sum(slR_t[r][:], maskedR[:], axis=X)
            nc.vector.reduce_sum(slI_t[r][:], maskedI[:], axis=X)
            # output-stack scale-mults for this round's k (overlaps the
            # next round's PE work instead of running as a serial tail)
            for c in range(CH):
                for kk, k in enumerate(ks):
                    col = c * nk + kk
                    slr = slR_t[r][:, col:col + 1]
                    sli = slI_t[r][:, col:col + 1]
                    kb = slice(k * OC, (k + 1) * OC)
                    kb2 = slice((K1 + k) * OC, (K1 + k + 1) * OC)
                    nc.vector.tensor_scalar(stk_r_t[c][:, kb], As_t[c][:, kb],
                                            slr, None, op0=Alu.mult)
                    nc.vector.tensor_scalar(stk_r_t[c][:, kb2],
                                            Bs_t[c][:, kb], sli, -1.0,
                                            op0=Alu.mult, op1=Alu.mult)
                    nc.vector.tensor_scalar(stk_i_t[c][:, kb], As_t[c][:, kb],
                                            sli, None, op0=Alu.mult)
                    nc.vector.tensor_scalar(stk_i_t[c][:, kb2],
                                            Bs_t[c][:, kb], slr, None,
                                            op0=Alu.mult)

    # ---------------- phase D: output reduces ----------------
    for c in range(CH):
        os_ = slice(c * OC, (c + 1) * OC)
        nc.vector.reduce_sum(
            or_sb[:, os_],
            stk_r_t[c][:].rearrange("p (s o) -> p o s", s=2 * K1), axis=X)
        nc.vector.reduce_sum(
            oi_sb[:, os_],
            stk_i_t[c][:].rearrange("p (s o) -> p o s", s=2 * K1), axis=X)
    nc.sync.dma_start(out_r.rearrange("(c p) o -> p c o", p=P),
                      or_sb[:].rearrange("p (c o) -> p c o", c=CH))
    nc.sync.dma_start(out_i.rearrange("(c p) o -> p c o", p=P),
                      oi_sb[:].rearrange("p (c o) -> p c o", c=CH))


def _host_prep(inputs):
    import ml_dtypes
    BF = ml_dtypes.bfloat16
    f = lambda k: np.ascontiguousarray(np.asarray(inputs[k], dtype=np.float32))
    X_real, X_imag = f("X_real"), f("X_imag")
    w_real, w_imag = f("w_real"), f("w_imag")
    aw_real, aw_imag = f("aw_real"), f("aw_imag")
    ab_real = float(np.asarray(inputs["ab_real"]))
    ab_imag = float(np.asarray(inputs["ab_imag"]))
    pa_real = float(np.asarray(inputs["pa_real"]))
    pa_imag = float(np.asarray(inputs["pa_imag"]))

    # bf16 + per-plane transpose of L: [B, K1, j, i]
    LrT = np.ascontiguousarray(
        np.asarray(inputs["L_real"], dtype=np.float32).astype(BF)
        .transpose(0, 1, 3, 2))
    LiT = np.ascontiguousarray(
        np.asarray(inputs["L_imag"], dtype=np.float32).astype(BF)
        .transpose(0, 1, 3, 2))

    ws_r, wd_r = aw_real[:C], aw_real[C:]
    ws_i, wd_i = aw_imag[:C], aw_imag[C:]
    w4 = np.stack([
        np.concatenate([ws_r, -ws_i]),
        np.concatenate([ws_i, ws_r]),
        np.concatenate([wd_r, -wd_i]),
        np.concatenate([wd_i, wd_r]),
    ], axis=1).astype(np.float32)                      # [128, 4]

    # wcat: rows 0:C = w_r^T (k-major cols), rows C:2C = w_i^T
    wr_t = w_real.transpose(1, 0, 2).reshape(C, K1 * OC)
    wi_t = w_imag.transpose(1, 0, 2).reshape(C, K1 * OC)
    wcat = np.concatenate([wr_t, wi_t], axis=0).astype(BF)

    pa_cols = np.stack([np.full(P, pa_real), np.full(P, pa_imag)],
                       axis=1).astype(np.float32)
    ab2 = np.array([[ab_real, ab_imag]], dtype=np.float32)

    in_maps = []
    for b in range(NCORES):
        in_maps.append({
            "x_real": X_real[b], "x_imag": X_imag[b],
            "l_realT": LrT[b], "l_imagT": LiT[b],
            "w4": w4, "wcat": wcat,
            "pa_cols": pa_cols, "ab2": ab2,
        })
    return in_maps


def kernel(**inputs):
    import os
    from concourse import bass_utils

    if "nc" not in _CACHE:
        _CACHE["nc"] = _build_nc()
    nc = _CACHE["nc"]
    in_maps = _host_prep(inputs)
    trace = os.environ.get("KERNEL_TRACE", "0") == "1"
    res = bass_utils.run_bass_kernel_spmd(nc, in_maps,
                                          core_ids=list(range(NCORES)),
                                          trace=trace)
    _CACHE["last_result"] = res
    out_r = np.stack([res.results[b]["out_r"] for b in range(NCORES)])
    out_i = np.stack([res.results[b]["out_i"] for b in range(NCORES)])
    return out_r, out_i



# revision 7
# speedup vs baseline: 1.1017x; 1.1017x over previous
"""Trainium2 Bass kernel for nn_ChebNet (complex Chebyshev GNN layer).

Sharding: data-parallel over batch B=8 across the 8 NeuronCores (one batch
element per core). No inter-core communication.

v3 design (vs 249us v2): full phase overlap + engine balance.

Structure (per core, batch b; N=1024, C=OC=64, K+1=5, j-chunks CH=8):
  era1: DMA weights/X; PE transposes X^T, attention projections, src
    broadcast rows, dst cols, A/B output stacks (bf16). L (bf16,
    host-pretiled [k][c][j][t][i]) prefetch starts at t=0.
  era2: attention per j-chunk in TRANSPOSED layout [j, i] (softmax over i
    = free axis): ACT chain Prelu x2 -> (Pool squares, Pool m2-add) ->
    Ln -> Exp(0.5 ln) = mag -> Exp(mag)+accum colsum -> er = Exp(mag -
    0.5 lnm2); DVE: d-stt, ar/ai stt (bf16), mai. Ln/mag/er ops merged
    over chunk PAIRS. One act table set (natural_log_exp) enforced by a
    post-compile fixup (the stock pass thrashes 0<->5 per chunk).
    Meanwhile PE runs diag-product passes k0,k1 gated per chunk.
  era3: PE passes k2..k4 at full clock. Per k-pass PSUM [128, 8ib, 2t,
    128] (4 banks, 2 passes in flight): per (c, ib) 3 matmuls:
    ar x (Lr|Li) -> (t0,t1); mai x Li -> t0; ai x Lr -> t1, so psum
    t0 = sum_j ar*Lr - ai*Li (= SLr diag block) and t1 = SLi. One
    start=True per bank (c==0, even ib) resets it; no zero-matmuls.
    Extraction per pass: ACT copy psum->bf16, DVE mask-mult (2x bf16),
    DVE reduce -> slAll[128, ib, t, k] strided. Tail per ib: Pool/DVE
    broadcast-tt Z = SL (x) A/B stacks -> strided reduce -> out DMA.
"""

import numpy as np
from contextlib import ExitStack

B, N, C, OC = 8, 1024, 64, 64
K1 = 5
P = 128
CH = N // P
NCORES = 8
EPS = 1e-12

_CACHE = {}


def _build_nc():
    import concourse.tile as tile
    from concourse import bacc, mybir

    f32 = mybir.dt.float32
    bf16 = mybir.dt.bfloat16
    Alu = mybir.AluOpType
    Act = mybir.ActivationFunctionType

    nc = bacc.Bacc("TRN2", target_bir_lowering=False, debug=False,
                   enable_asserts=False, num_devices=NCORES)

    x_real = nc.dram_tensor("x_real", [N, C], f32, kind="ExternalInput").ap()
    x_imag = nc.dram_tensor("x_imag", [N, C], f32, kind="ExternalInput").ap()
    # host-pretiled: [k][c][j 128][t 2][i 1024], t0=Lr^T, t1=Li^T (bf16)
    lcat = nc.dram_tensor("lcat", [K1, CH, P, 2, N], bf16,
                          kind="ExternalInput").ap()
    w4 = nc.dram_tensor("w4", [2 * C, 4], f32, kind="ExternalInput").ap()
    wcat = nc.dram_tensor("wcat", [2 * C, K1 * OC], bf16,
                          kind="ExternalInput").ap()
    pa_cols = nc.dram_tensor("pa_cols", [P, 2], f32, kind="ExternalInput").ap()
    ab2 = nc.dram_tensor("ab2", [1, 2], f32, kind="ExternalInput").ap()
    out_r = nc.dram_tensor("out_r", [N, OC], f32, kind="ExternalOutput").ap()
    out_i = nc.dram_tensor("out_i", [N, OC], f32, kind="ExternalOutput").ap()

    with tile.TileContext(nc) as tc:
        with ExitStack() as ctx:
            _emit(ctx, tc, nc, mybir, f32, bf16, Alu, Act,
                  x_real, x_imag, lcat, w4, wcat, pa_cols, ab2, out_r, out_i)
    nc.compile()

    # --- act-table fixup: the stock placement alternates between table
    # sets 0 (exp) and 5 (ln) every chunk (~1.3us per reload). Set 6
    # (natural_log_exp_and_others) serves every function used here
    # (parametric_relu, square, ln, exp, copy), so keep one load of it.
    nloads = 0
    for b in nc.main_func.blocks:
        keep = []
        for inst in b.instructions:
            if isinstance(inst, mybir.InstLoadActFuncSet):
                nloads += 1
                if nloads == 1:
                    inst.act_func_set_id = 6
                    keep.append(inst)
                continue
            keep.append(inst)
        b.instructions[:] = keep
    return nc


def _emit(ctx, tc, nc, mybir, f32, bf16, Alu, Act,
          x_real, x_imag, lcat, w4, wcat, pa_cols, ab2, out_r, out_i):
    X = mybir.AxisListType.X

    const = ctx.enter_context(tc.tile_pool(name="const", bufs=1))
    persist = ctx.enter_context(tc.tile_pool(name="persist", bufs=1))

    ident_i = const.tile([P, P], mybir.dt.int32)
    nc.gpsimd.iota(ident_i[:], pattern=[[1, P]], base=0, channel_multiplier=-1)
    ident = const.tile([P, P], f32)
    nc.vector.tensor_scalar(ident[:], ident_i[:], 0, None, op0=Alu.is_equal)
    mask_bf = const.tile([P, P], bf16)
    nc.vector.tensor_scalar(mask_bf[:], ident_i[:], 0, None, op0=Alu.is_equal)

    ones_row = const.tile([1, P], f32)
    nc.vector.memset(ones_row[:], 1.0)
    eps_col = const.tile([P, 1], f32)
    nc.vector.memset(eps_col[:], EPS)

    w4_sb = const.tile([P, 4], f32)
    nc.sync.dma_start(w4_sb[:], w4[:])
    wcat_sb = const.tile([P, K1 * OC], bf16)
    nc.sync.dma_start(wcat_sb[:], wcat[:])
    pa_sb = const.tile([P, 2], f32)
    nc.sync.dma_start(pa_sb[:], pa_cols[:])
    ab_sb = const.tile([1, 2], f32)
    nc.sync.dma_start(ab_sb[:], ab2[:])

    # ---------------- persistent tiles ----------------
    xt = persist.tile([P, N], f32)         # [Xr^T; Xi^T]
    xtb = persist.tile([P, N], bf16)
    src_bc_r = persist.tile([P, N], f32)
    src_bc_i = persist.tile([P, N], f32)
    dstT = persist.tile([P, 2 * CH], f32)

    a_pool = ctx.enter_context(tc.tile_pool(name="apool", bufs=3 * CH))
    ar_t, ai_t, mai_t = [], [], []
    for c in range(CH):
        ar_t.append(a_pool.tile([P, N], bf16, tag="apool", name=f"ar{c}"))
        ai_t.append(a_pool.tile([P, N], bf16, tag="apool", name=f"ai{c}"))
        mai_t.append(a_pool.tile([P, N], bf16, tag="apool", name=f"mai{c}"))

    ab_stack = ctx.enter_context(tc.tile_pool(name="abst", bufs=2 * CH))
    As_t, Bs_t = [], []
    for c in range(CH):
        As_t.append(ab_stack.tile([P, K1, OC], bf16, tag="abst", name=f"As{c}"))
        Bs_t.append(ab_stack.tile([P, K1, OC], bf16, tag="abst", name=f"Bs{c}"))

    # L tiles: rotating pool, one tile per (k, c) = [128j, 2t, 1024i] bf16
    # (DMAs emitted after the phase-A loads so X isn't queued behind 20MB)
    l_pool = ctx.enter_context(tc.tile_pool(name="lpool", bufs=10))
    l_tiles = {}

    # SL results [p, ib, t(R/I), k] fp32
    slAll = persist.tile([P, CH, 2, K1], f32)
    cs_cols = persist.tile([P, CH], f32)     # per-chunk softmax colsums
    inv_cols = persist.tile([P, CH], f32)

    # ---------------- era 1: transposes / projections / stacks ----------
    pa_ctx = ExitStack()
    pa_pool = pa_ctx.enter_context(tc.tile_pool(name="phaseA", bufs=1))
    with tc.tile_pool(name="pa_ps", bufs=2, space="PSUM") as pa_ps, \
         tc.tile_pool(name="pa_proj", bufs=3, space="PSUM") as pa_proj:
        xr_sb = pa_pool.tile([P, CH, C], f32)
        xi_sb = pa_pool.tile([P, CH, C], f32)
        nc.sync.dma_start(xr_sb[:], x_real.rearrange("(c p) d -> p c d", p=P))
        nc.sync.dma_start(xi_sb[:], x_imag.rearrange("(c p) d -> p c d", p=P))

        # L prefetch: emitted now (after X/weight loads) so it starts at
        # t~0 without delaying the phase-A inputs.
        for k in range(K1):
            for c in range(CH):
                lt = l_pool.tile([P, 2, N], bf16, tag="lpool",
                                 name=f"L{k}_{c}")
                l_tiles[(k, c)] = lt
                nc.sync.dma_start(lt[:], lcat[k, c])

        for c in range(CH):
            tp = pa_ps.tile([C, P], f32, tag="tp")
            nc.tensor.transpose(tp[:], xr_sb[:, c, :], ident[:])
            nc.vector.tensor_copy(xt[0:C, c * P:(c + 1) * P], tp[:])
            tp2 = pa_ps.tile([C, P], f32, tag="tp")
            nc.tensor.transpose(tp2[:], xi_sb[:, c, :], ident[:])
            nc.scalar.copy(xt[C:2 * C, c * P:(c + 1) * P], tp2[:])

        nc.vector.tensor_copy(xtb[:], xt[:])

        srcr_sb = pa_pool.tile([1, N], f32)
        srci_sb = pa_pool.tile([1, N], f32)
        dst_sb = pa_pool.tile([2, N], f32)
        for h in range(2):
            hs = slice(h * 512, (h + 1) * 512)
            srcr_ps = pa_proj.tile([1, 512], f32, tag="proj")
            nc.tensor.matmul(srcr_ps[:], w4_sb[:, 0:1], xt[:, hs],
                             start=True, stop=True)
            nc.scalar.copy(srcr_sb[:, hs], srcr_ps[:])
            srci_ps = pa_proj.tile([1, 512], f32, tag="proj")
            nc.tensor.matmul(srci_ps[:], w4_sb[:, 1:2], xt[:, hs],
                             start=True, stop=True)
            nc.scalar.copy(srci_sb[:, hs], srci_ps[:])
            dst_ps = pa_proj.tile([2, 512], f32, tag="proj")
            nc.tensor.matmul(dst_ps[:], w4_sb[:, 2:4], xt[:, hs],
                             start=True, stop=True)
            nc.scalar.copy(dst_sb[:, hs], dst_ps[:])
        nc.vector.tensor_scalar(srcr_sb[:], srcr_sb[:], ab_sb[0:1, 0:1],
                                None, op0=Alu.add)
        nc.vector.tensor_scalar(srci_sb[:], srci_sb[:], ab_sb[0:1, 1:2],
                                None, op0=Alu.add)

    with tc.tile_pool(name="pa_bc", bufs=2, space="PSUM") as pa_bc, \
         tc.tile_pool(name="pa_tpd", bufs=2, space="PSUM") as pa_tpd, \
         tc.tile_pool(name="pa_ab", bufs=2, space="PSUM") as pa_ab:
        for row_sb, dstt in ((srcr_sb, src_bc_r), (srci_sb, src_bc_i)):
            for h in range(2):
                hs = slice(h * 512, (h + 1) * 512)
                bc = pa_bc.tile([P, 512], f32, tag="srcbc")
                nc.tensor.matmul(bc[:], ones_row[:], row_sb[:, hs],
                                 start=True, stop=True)
                if h == 0:
                    nc.vector.tensor_copy(dstt[:, hs], bc[:])
                else:
                    nc.scalar.copy(dstt[:, hs], bc[:])

        for c in range(CH):
            tp3 = pa_tpd.tile([P, 2], f32, tag="tpd")
            nc.tensor.transpose(tp3[:], dst_sb[:, c * P:(c + 1) * P],
                                ident[0:2, 0:2])
            nc.vector.tensor_copy(dstT[:, 2 * c:2 * c + 2], tp3[:])

        for c in range(CH):
            cs = slice(c * P, (c + 1) * P)
            psA = pa_ab.tile([P, K1 * OC], f32, tag="psAB")
            nc.tensor.matmul(psA[:], xtb[0:C, cs], wcat_sb[0:C, :],
                             start=True, stop=True)
            if c % 2 == 0:
                nc.scalar.copy(As_t[c][:].rearrange("p a b -> p (a b)"), psA[:])
            else:
                nc.vector.tensor_copy(
                    As_t[c][:].rearrange("p a b -> p (a b)"), psA[:])
            psB = pa_ab.tile([P, K1 * OC], f32, tag="psAB")
            nc.tensor.matmul(psB[:], xtb[C:2 * C, cs], wcat_sb[C:2 * C, :],
                             start=True, stop=True)
            if c % 2 == 0:
                nc.scalar.copy(Bs_t[c][:].rearrange("p a b -> p (a b)"), psB[:])
            else:
                nc.vector.tensor_copy(
                    Bs_t[c][:].rearrange("p a b -> p (a b)"), psB[:])

    pa_ctx.close()  # free phase-A SBUF (xr/xi/src/dst staging)

    # ---------------- era 2: attention (pair-merged ACT chain) ----------
    st_pool = ctx.enter_context(tc.tile_pool(name="stp", bufs=4))
    sq_pool = ctx.enter_context(tc.tile_pool(name="sqp", bufs=2))
    pairA = ctx.enter_context(tc.tile_pool(name="pairA", bufs=1))
    pairB = ctx.enter_context(tc.tile_pool(name="pairB", bufs=2))
    e_pool = ctx.enter_context(tc.tile_pool(name="epool", bufs=2))

    m2p = pairA.tile([P, 2, N], f32, name="m2p")
    lnm2p = pairA.tile([P, 2, N], f32, name="lnm2p")
    magp = pairA.tile([P, 2, N], f32, name="magp")
    dp = pairA.tile([P, 2, N], f32, name="dp")

    for pair in range(CH // 2):
        erp = pairB.tile([P, 2, N], bf16, tag="erp", name=f"erp{pair}")
        sT = {}
        for h in range(2):
            c = 2 * pair + h
            sTr = st_pool.tile([P, N], f32, tag="stp", name=f"sTr{c}")
            nc.scalar.activation(sTr[:], src_bc_r[:], Act.Prelu,
                                 bias=dstT[:, 2 * c:2 * c + 1],
                                 alpha=pa_sb[:, 0:1])
            sTi = st_pool.tile([P, N], f32, tag="stp", name=f"sTi{c}")
            nc.scalar.activation(sTi[:], src_bc_i[:], Act.Prelu,
                                 bias=dstT[:, 2 * c + 1:2 * c + 2],
                                 alpha=pa_sb[:, 1:2])
            sT[h] = (sTr, sTi)
            sqr = sq_pool.tile([P, N], f32, tag="sqp", name=f"sqr{c}")
            nc.gpsimd.tensor_tensor(sqr[:], sTr[:], sTr[:], op=Alu.mult)
            sqi = sq_pool.tile([P, N], f32, tag="sqp", name=f"sqi{c}")
            nc.gpsimd.tensor_tensor(sqi[:], sTi[:], sTi[:], op=Alu.mult)
            nc.gpsimd.tensor_tensor(m2p[:, h, :], sqr[:], sqi[:], op=Alu.add)

        nc.scalar.activation(lnm2p[:].rearrange("p a b -> p (a b)"),
                             m2p[:].rearrange("p a b -> p (a b)"),
                             Act.Ln, bias=eps_col[:, 0:1])
        nc.scalar.activation(magp[:].rearrange("p a b -> p (a b)"),
                             lnm2p[:].rearrange("p a b -> p (a b)"),
                             Act.Exp, scale=0.5)
        for h in range(2):
            c = 2 * pair + h
            e_scr = e_pool.tile([P, N], bf16, tag="epool", name=f"e{c}")
            nc.scalar.activation(e_scr[:], magp[:, h, :], Act.Exp,
                                 accum_out=cs_cols[:, c:c + 1])
        nc.vector.scalar_tensor_tensor(dp[:].rearrange("p a b -> p (a b)"),
                                       lnm2p[:].rearrange("p a b -> p (a b)"),
                                       -0.5,
                                       magp[:].rearrange("p a b -> p (a b)"),
                                       op0=Alu.mult, op1=Alu.add)
        nc.scalar.activation(erp[:].rearrange("p a b -> p (a b)"),
                             dp[:].rearrange("p a b -> p (a b)"), Act.Exp)
        for h in range(2):
            c = 2 * pair + h
            sTr, sTi = sT[h]
            nc.vector.reciprocal(inv_cols[:, c:c + 1], cs_cols[:, c:c + 1])
            nc.vector.scalar_tensor_tensor(ar_t[c][:], sTr[:],
                                           inv_cols[:, c:c + 1], erp[:, h, :],
                                           op0=Alu.mult, op1=Alu.mult)
            nc.vector.scalar_tensor_tensor(ai_t[c][:], sTi[:],
                                           inv_cols[:, c:c + 1], erp[:, h, :],
                                           op0=Alu.mult, op1=Alu.mult)
            nc.vector.tensor_scalar(mai_t[c][:], ai_t[c][:], -1.0, None,
                                    op0=Alu.mult)

    # ---------------- era 2/3: PE diag passes + extraction + tail -------
    cp_pool = ctx.enter_context(tc.tile_pool(name="cpp", bufs=2))
    z_pool = ctx.enter_context(tc.tile_pool(name="zp", bufs=4))
    msl_pool = ctx.enter_context(tc.tile_pool(name="mslp", bufs=2))
    o_pool = ctx.enter_context(tc.tile_pool(name="op", bufs=4))

    with tc.tile_pool(name="diag_ps", bufs=2, space="PSUM") as dps:
        for k in range(K1):
            ps = dps.tile([P, CH, 2, P], f32, tag="dps", name=f"ps{k}")
            for c in range(CH):
                lt = l_tiles[(k, c)]
                last = (c == CH - 1)
                for ib in range(CH):
                    ibs = slice(ib * P, (ib + 1) * P)
                    # mm1: ar x (Lr|Li) -> (t0 += ar*Lr, t1 += ar*Li)
                    nc.tensor.matmul(ps[:, ib, :, :], ar_t[c][:, ibs],
                                     lt[:, :, ibs],
                                     start=(c == 0 and ib % 2 == 0),
                                     stop=False, skip_group_check=True)
                    # mm2: (-ai) x Li -> t0  (=> t0 = sum ar*Lr - ai*Li)
                    nc.tensor.matmul(ps[:, ib, 0, :], mai_t[c][:, ibs],
                                     lt[:, 1, ibs], start=False, stop=last,
                                     skip_group_check=True)
                    # mm3: ai x Lr -> t1     (=> t1 = sum ar*Li + ai*Lr)
                    nc.tensor.matmul(ps[:, ib, 1, :], ai_t[c][:, ibs],
                                     lt[:, 0, ibs], start=False, stop=last,
                                     skip_group_check=True)
            # extraction: ACT psum->bf16, DVE mask-mult (2x), DVE reduce
            cpb = cp_pool.tile([P, CH * 2, P], bf16, tag="cpp", name=f"cp{k}")
            nc.scalar.activation(cpb[:].rearrange("p a b -> p (a b)"),
                                 ps[:].rearrange("p a b c -> p (a b c)"),
                                 Act.Copy)
            mskd = cp_pool.tile([P, CH * 2, P], bf16, tag="cpp",
                                name=f"mk{k}")
            mask_bc = mask_bf[:].unsqueeze(1).broadcast_to([P, CH * 2, P])
            nc.vector.tensor_tensor(mskd[:], cpb[:], mask_bc, op=Alu.mult)
            nc.vector.reduce_sum(
                slAll[:, :, :, k].rearrange("p a b -> p (a b)"),
                mskd[:], axis=X)

    # tail per ib: Z = SL (x) A/B stacks, strided reduce, out DMA
    for ib in range(CH):
        slr = slAll[:, ib, 0, :]                      # [128, 5]
        sli = slAll[:, ib, 1, :]
        msli = msl_pool.tile([P, K1], f32, tag="mslp", name=f"msli{ib}")
        nc.vector.tensor_scalar(msli[:], sli, -1.0, None, op0=Alu.mult)
        slr_bc = slr.unsqueeze(2).broadcast_to([P, K1, OC])
        sli_bc = sli.unsqueeze(2).broadcast_to([P, K1, OC])
        msli_bc = msli[:].unsqueeze(2).broadcast_to([P, K1, OC])

        zr = z_pool.tile([P, 2, K1, OC], bf16, tag="zp", name=f"zr{ib}")
        zi = z_pool.tile([P, 2, K1, OC], bf16, tag="zp", name=f"zi{ib}")
        nc.gpsimd.tensor_tensor(zr[:, 0, :, :], As_t[ib][:], slr_bc,
                                op=Alu.mult)
        nc.gpsimd.tensor_tensor(zr[:, 1, :, :], Bs_t[ib][:], msli_bc,
                                op=Alu.mult)
        nc.vector.tensor_tensor(zi[:, 0, :, :], As_t[ib][:], sli_bc,
                                op=Alu.mult)
        nc.vector.tensor_tensor(zi[:, 1, :, :], Bs_t[ib][:], slr_bc,
                                op=Alu.mult)
        orc = o_pool.tile([P, OC], f32, tag="op", name=f"or{ib}")
        oic = o_pool.tile([P, OC], f32, tag="op", name=f"oi{ib}")
        nc.vector.reduce_sum(
            orc[:], zr[:].rearrange("p s k o -> p o (s k)"), axis=X)
        nc.vector.reduce_sum(
            oic[:], zi[:].rearrange("p s k o -> p o (s k)"), axis=X)
        nc.sync.dma_start(out_r[ib * P:(ib + 1) * P, :], orc[:])
        nc.sync.dma_start(out_i[ib * P:(ib + 1) * P, :], oic[:])


def _host_prep(inputs):
    import ml_dtypes
    BF = ml_dtypes.bfloat16
    f = lambda k: np.ascontiguousarray(np.asarray(inputs[k], dtype=np.float32))
    X_real, X_imag = f("X_real"), f("X_imag")
    w_real, w_imag = f("w_real"), f("w_imag")
    aw_real, aw_imag = f("aw_real"), f("aw_imag")
    ab_real = float(np.asarray(inputs["ab_real"]))
    ab_imag = float(np.asarray(inputs["ab_imag"]))
    pa_real = float(np.asarray(inputs["pa_real"]))
    pa_imag = float(np.asarray(inputs["pa_imag"]))

    # lcat[b]: [k][c][j 128][t 2][i 1024] with t0 = Lr^T, t1 = Li^T
    Lr = np.asarray(inputs["L_real"], dtype=np.float32)
    Li = np.asarray(inputs["L_imag"], dtype=np.float32)
    LrT = Lr.transpose(0, 1, 3, 2).reshape(B, K1, CH, P, N)
    LiT = Li.transpose(0, 1, 3, 2).reshape(B, K1, CH, P, N)
    lcat = np.ascontiguousarray(
        np.stack([LrT, LiT], axis=4).astype(BF))     # [B, K1, CH, P, 2, N]

    ws_r, wd_r = aw_real[:C], aw_real[C:]
    ws_i, wd_i = aw_imag[:C], aw_imag[C:]
    w4 = np.stack([
        np.concatenate([ws_r, -ws_i]),
        np.concatenate([ws_i, ws_r]),
        np.concatenate([wd_r, -wd_i]),
        np.concatenate([wd_i, wd_r]),
    ], axis=1).astype(np.float32)

    wr_t = w_real.transpose(1, 0, 2).reshape(C, K1 * OC)
    wi_t = w_imag.transpose(1, 0, 2).reshape(C, K1 * OC)
    wcat = np.concatenate([wr_t, wi_t], axis=0).astype(BF)

    pa_cols = np.stack([np.full(P, pa_real), np.full(P, pa_imag)],
                       axis=1).astype(np.float32)
    ab2 = np.array([[ab_real, ab_imag]], dtype=np.float32)

    in_maps = []
    for b in range(NCORES):
        in_maps.append({
            "x_real": X_real[b], "x_imag": X_imag[b],
            "lcat": lcat[b],
            "w4": w4, "wcat": wcat,
            "pa_cols": pa_cols, "ab2": ab2,
        })
    return in_maps


def kernel(**inputs):
    import os
    from concourse import bass_utils

    if "nc" not in _CACHE:
        _CACHE["nc"] = _build_nc()
    nc = _CACHE["nc"]
    in_maps = _host_prep(inputs)
    trace = os.environ.get("KERNEL_TRACE", "0") == "1"
    res = bass_utils.run_bass_kernel_spmd(nc, in_maps,
                                          core_ids=list(range(NCORES)),
                                          trace=trace)
    _CACHE["last_result"] = res
    out_r = np.stack([res.results[b]["out_r"] for b in range(NCORES)])
    out_i = np.stack([res.results[b]["out_i"] for b in range(NCORES)])
    return out_r, out_i


# revision 13
# speedup vs baseline: 1.2139x; 1.1019x over previous
"""Trainium2 Bass kernel for nn_ChebNet (complex Chebyshev GNN layer).

Sharding: data-parallel over batch B=8 across the 8 NeuronCores (one batch
element per core). No inter-core communication.

v3 design (vs 249us v2): full phase overlap + engine balance.

Structure (per core, batch b; N=1024, C=OC=64, K+1=5, j-chunks CH=8):
  era1: DMA weights/X; PE transposes X^T, attention projections, src
    broadcast rows, dst cols, A/B output stacks (bf16). L (bf16,
    host-pretiled [k][c][j][t][i]) prefetch starts at t=0.
  era2: attention per j-chunk in TRANSPOSED layout [j, i] (softmax over i
    = free axis): ACT chain Prelu x2 -> (Pool squares, Pool m2-add) ->
    Ln -> Exp(0.5 ln) = mag -> Exp(mag)+accum colsum -> er = Exp(mag -
    0.5 lnm2); DVE: d-stt, ar/ai stt (bf16), mai. Ln/mag/er ops merged
    over chunk PAIRS. One act table set (natural_log_exp) enforced by a
    post-compile fixup (the stock pass thrashes 0<->5 per chunk).
    Meanwhile PE runs diag-product passes k0,k1 gated per chunk.
  era3: PE passes k2..k4 at full clock. Per k-pass PSUM [128, 8ib, 2t,
    128] (4 banks, 2 passes in flight): per (c, ib) 3 matmuls:
    ar x (Lr|Li) -> (t0,t1); mai x Li -> t0; ai x Lr -> t1, so psum
    t0 = sum_j ar*Lr - ai*Li (= SLr diag block) and t1 = SLi. One
    start=True per bank (c==0, even ib) resets it; no zero-matmuls.
    Extraction per pass: ACT copy psum->bf16, DVE mask-mult (2x bf16),
    DVE reduce -> slAll[128, ib, t, k] strided. Tail per ib: Pool/DVE
    broadcast-tt Z = SL (x) A/B stacks -> strided reduce -> out DMA.
"""

import numpy as np
from contextlib import ExitStack

B, N, C, OC = 8, 1024, 64, 64
K1 = 5
P = 128
CH = N // P
NCORES = 8
EPS = 1e-12

_CACHE = {}


def _build_nc():
    import concourse.tile as tile
    from concourse import bacc, mybir

    f32 = mybir.dt.float32
    bf16 = mybir.dt.bfloat16
    Alu = mybir.AluOpType
    Act = mybir.ActivationFunctionType

    nc = bacc.Bacc("TRN2", target_bir_lowering=False, debug=False,
                   enable_asserts=False, num_devices=NCORES)

    x_real = nc.dram_tensor("x_real", [N, C], f32, kind="ExternalInput").ap()
    x_imag = nc.dram_tensor("x_imag", [N, C], f32, kind="ExternalInput").ap()
    # host-pretiled: [k][c][j 128][t 2][i 1024], t0=Lr^T, t1=Li^T (bf16)
    lcat = nc.dram_tensor("lcat", [K1, CH, P, 2, N], bf16,
                          kind="ExternalInput").ap()
    w4 = nc.dram_tensor("w4", [2 * C, 4], f32, kind="ExternalInput").ap()
    wcat = nc.dram_tensor("wcat", [2 * C, K1 * OC], bf16,
                          kind="ExternalInput").ap()
    pa_cols = nc.dram_tensor("pa_cols", [P, 2], f32, kind="ExternalInput").ap()
    ab2 = nc.dram_tensor("ab2", [1, 2], f32, kind="ExternalInput").ap()
    out_r = nc.dram_tensor("out_r", [N, OC], f32, kind="ExternalOutput").ap()
    out_i = nc.dram_tensor("out_i", [N, OC], f32, kind="ExternalOutput").ap()

    with tile.TileContext(nc) as tc:
        with ExitStack() as ctx:
            _emit(ctx, tc, nc, mybir, f32, bf16, Alu, Act,
                  x_real, x_imag, lcat, w4, wcat, pa_cols, ab2, out_r, out_i)
    nc.compile()

    # --- act-table fixup: the stock placement alternates between table
    # sets 0 (exp) and 5 (ln) every chunk (~1.3us per reload). Set 6
    # (natural_log_exp_and_others) serves every function used here
    # (parametric_relu, square, ln, exp, copy), so keep one load of it.
    nloads = 0
    for b in nc.main_func.blocks:
        keep = []
        for inst in b.instructions:
            if isinstance(inst, mybir.InstLoadActFuncSet):
                nloads += 1
                if nloads == 1:
                    inst.act_func_set_id = 6
                    keep.append(inst)
                continue
            keep.append(inst)
        b.instructions[:] = keep
    return nc


def _emit(ctx, tc, nc, mybir, f32, bf16, Alu, Act,
          x_real, x_imag, lcat, w4, wcat, pa_cols, ab2, out_r, out_i):
    X = mybir.AxisListType.X

    const = ctx.enter_context(tc.tile_pool(name="const", bufs=1))
    persist = ctx.enter_context(tc.tile_pool(name="persist", bufs=1))

    ident_i = const.tile([P, P], mybir.dt.int32)
    nc.gpsimd.iota(ident_i[:], pattern=[[1, P]], base=0, channel_multiplier=-1)
    ident = const.tile([P, P], f32)
    nc.vector.tensor_scalar(ident[:], ident_i[:], 0, None, op0=Alu.is_equal)
    mask_bf = const.tile([P, P], bf16)
    nc.vector.tensor_scalar(mask_bf[:], ident_i[:], 0, None, op0=Alu.is_equal)

    ones_row = const.tile([1, P], f32)
    nc.vector.memset(ones_row[:], 1.0)
    eps_col = const.tile([P, 1], f32)
    nc.vector.memset(eps_col[:], EPS)

    w4_sb = const.tile([P, 4], f32)
    nc.sync.dma_start(w4_sb[:], w4[:])
    wcat_sb = const.tile([P, K1 * OC], bf16)
    nc.sync.dma_start(wcat_sb[:], wcat[:])
    pa_sb = const.tile([P, 2], f32)
    nc.sync.dma_start(pa_sb[:], pa_cols[:])
    ab_sb = const.tile([1, 2], f32)
    nc.sync.dma_start(ab_sb[:], ab2[:])

    # ---------------- persistent tiles ----------------
    xt = persist.tile([P, N], f32)         # [Xr^T; Xi^T]
    xtb = persist.tile([P, N], bf16)
    src_bc_r = persist.tile([P, N], f32)
    src_bc_i = persist.tile([P, N], f32)
    dstT = persist.tile([P, 2 * CH], f32)

    a_pool = ctx.enter_context(tc.tile_pool(name="apool", bufs=3 * CH))
    ar_t, ai_t, mai_t = [], [], []
    for c in range(CH):
        ar_t.append(a_pool.tile([P, N], bf16, tag="apool", name=f"ar{c}"))
        ai_t.append(a_pool.tile([P, N], bf16, tag="apool", name=f"ai{c}"))
        mai_t.append(a_pool.tile([P, N], bf16, tag="apool", name=f"mai{c}"))

    ab_stack = ctx.enter_context(tc.tile_pool(name="abst", bufs=2 * CH))
    As_t, Bs_t = [], []
    for c in range(CH):
        As_t.append(ab_stack.tile([P, K1, OC], bf16, tag="abst", name=f"As{c}"))
        Bs_t.append(ab_stack.tile([P, K1, OC], bf16, tag="abst", name=f"Bs{c}"))

    # L tiles: rotating pool, one tile per (k, c) = [128j, 2t, 1024i] bf16
    # (DMAs emitted after the phase-A loads so X isn't queued behind 20MB)
    l_pool = ctx.enter_context(tc.tile_pool(name="lpool", bufs=9))
    l_tiles = {}

    # SL results [p, ib, t(R/I), k] fp32
    slAll = persist.tile([P, CH, 2, K1], f32)
    cs_cols = persist.tile([P, CH], f32)     # per-chunk softmax colsums
    inv_cols = persist.tile([P, CH], f32)

    # ---------------- era 1: transposes / projections / stacks ----------
    pa_ctx = ExitStack()
    pa_pool = pa_ctx.enter_context(tc.tile_pool(name="phaseA", bufs=1))
    with tc.tile_pool(name="pa_ps", bufs=2, space="PSUM") as pa_ps, \
         tc.tile_pool(name="pa_proj", bufs=3, space="PSUM") as pa_proj:
        xr_sb = pa_pool.tile([P, CH, C], f32)
        xi_sb = pa_pool.tile([P, CH, C], f32)
        nc.sync.dma_start(xr_sb[:], x_real.rearrange("(c p) d -> p c d", p=P))
        nc.sync.dma_start(xi_sb[:], x_imag.rearrange("(c p) d -> p c d", p=P))

        # L prefetch: emitted now (after X/weight loads) so it starts at
        # t~0 without delaying the phase-A inputs.
        for k in range(K1):
            for c in range(CH):
                lt = l_pool.tile([P, 2, N], bf16, tag="lpool",
                                 name=f"L{k}_{c}")
                l_tiles[(k, c)] = lt
                nc.sync.dma_start(lt[:], lcat[k, c])

        # all 8 transposes per r/i go into one psum tile -> ONE copy each.
        # tile is 2 banks (1024 f32); start=True resets a whole bank, so
        # flag it only on the first write of each bank (c==0, c==4).
        tpr = pa_ps.tile([C, CH, P], f32, tag="tp")
        for c in range(CH):
            nc.tensor.matmul(tpr[:, c, :], xr_sb[:, c, :], ident[:],
                             is_transpose=True, start=(c in (0, 4)),
                             stop=(c in (3, 7)), skip_group_check=True)
        nc.vector.tensor_copy(xt[0:C, :], tpr[:].rearrange("p a b -> p (a b)"))
        tpi = pa_ps.tile([C, CH, P], f32, tag="tp")
        for c in range(CH):
            nc.tensor.matmul(tpi[:, c, :], xi_sb[:, c, :], ident[:],
                             is_transpose=True, start=(c in (0, 4)),
                             stop=(c in (3, 7)), skip_group_check=True)
        nc.scalar.copy(xt[C:2 * C, :], tpi[:].rearrange("p a b -> p (a b)"))

        nc.vector.tensor_copy(xtb[:], xt[:])

        srcr_sb = pa_pool.tile([1, N], f32)
        srci_sb = pa_pool.tile([1, N], f32)
        dst_sb = pa_pool.tile([2, N], f32)
        for h in range(2):
            hs = slice(h * 512, (h + 1) * 512)
            srcr_ps = pa_proj.tile([1, 512], f32, tag="proj")
            nc.tensor.matmul(srcr_ps[:], w4_sb[:, 0:1], xt[:, hs],
                             start=True, stop=True)
            nc.scalar.copy(srcr_sb[:, hs], srcr_ps[:])
            srci_ps = pa_proj.tile([1, 512], f32, tag="proj")
            nc.tensor.matmul(srci_ps[:], w4_sb[:, 1:2], xt[:, hs],
                             start=True, stop=True)
            nc.scalar.copy(srci_sb[:, hs], srci_ps[:])
            dst_ps = pa_proj.tile([2, 512], f32, tag="proj")
            nc.tensor.matmul(dst_ps[:], w4_sb[:, 2:4], xt[:, hs],
                             start=True, stop=True)
            nc.scalar.copy(dst_sb[:, hs], dst_ps[:])
        nc.vector.tensor_scalar(srcr_sb[:], srcr_sb[:], ab_sb[0:1, 0:1],
                                None, op0=Alu.add)
        nc.vector.tensor_scalar(srci_sb[:], srci_sb[:], ab_sb[0:1, 1:2],
                                None, op0=Alu.add)

    with tc.tile_pool(name="pa_bc", bufs=2, space="PSUM") as pa_bc, \
         tc.tile_pool(name="pa_tpd", bufs=2, space="PSUM") as pa_tpd, \
         tc.tile_pool(name="pa_ab", bufs=2, space="PSUM") as pa_ab:
        for row_sb, dstt in ((srcr_sb, src_bc_r), (srci_sb, src_bc_i)):
            for h in range(2):
                hs = slice(h * 512, (h + 1) * 512)
                bc = pa_bc.tile([P, 512], f32, tag="srcbc")
                nc.tensor.matmul(bc[:], ones_row[:], row_sb[:, hs],
                                 start=True, stop=True)
                if h == 0:
                    nc.vector.tensor_copy(dstt[:, hs], bc[:])
                else:
                    nc.scalar.copy(dstt[:, hs], bc[:])

        for c in range(CH):
            tp3 = pa_tpd.tile([P, 2], f32, tag="tpd")
            nc.tensor.transpose(tp3[:], dst_sb[:, c * P:(c + 1) * P],
                                ident[0:2, 0:2])
            nc.vector.tensor_copy(dstT[:, 2 * c:2 * c + 2], tp3[:])

        for c in range(CH):
            cs = slice(c * P, (c + 1) * P)
            psA = pa_ab.tile([P, K1 * OC], f32, tag="psAB")
            nc.tensor.matmul(psA[:], xtb[0:C, cs], wcat_sb[0:C, :],
                             start=True, stop=True)
            if c % 2 == 0:
                nc.scalar.copy(As_t[c][:].rearrange("p a b -> p (a b)"), psA[:])
            else:
                nc.vector.tensor_copy(
                    As_t[c][:].rearrange("p a b -> p (a b)"), psA[:])
            psB = pa_ab.tile([P, K1 * OC], f32, tag="psAB")
            nc.tensor.matmul(psB[:], xtb[C:2 * C, cs], wcat_sb[C:2 * C, :],
                             start=True, stop=True)
            if c % 2 == 0:
                nc.scalar.copy(Bs_t[c][:].rearrange("p a b -> p (a b)"), psB[:])
            else:
                nc.vector.tensor_copy(
                    Bs_t[c][:].rearrange("p a b -> p (a b)"), psB[:])

    pa_ctx.close()  # free phase-A SBUF (xr/xi/src/dst staging)

    # ---------------- era 2: attention (pair-merged ACT chain) ----------
    st_pool = ctx.enter_context(tc.tile_pool(name="stp", bufs=6))
    sq_pool = ctx.enter_context(tc.tile_pool(name="sqp", bufs=2))
    pairA = ctx.enter_context(tc.tile_pool(name="pairA", bufs=1))
    pairB = ctx.enter_context(tc.tile_pool(name="pairB", bufs=2))
    e_pool = ctx.enter_context(tc.tile_pool(name="epool", bufs=1))

    m2p = pairA.tile([P, 2, N], f32, name="m2p")
    lnm2p = pairA.tile([P, 2, N], f32, name="lnm2p")
    magp = pairA.tile([P, 2, N], f32, name="magp")
    dp = pairA.tile([P, 2, N], f32, name="dp")

    for pair in range(CH // 2):
        erp = pairB.tile([P, 2, N], bf16, tag="erp", name=f"erp{pair}")
        sT = {}
        for h in range(2):
            c = 2 * pair + h
            sTr = st_pool.tile([P, N], f32, tag="stp", name=f"sTr{c}")
            nc.scalar.activation(sTr[:], src_bc_r[:], Act.Prelu,
                                 bias=dstT[:, 2 * c:2 * c + 1],
                                 alpha=pa_sb[:, 0:1])
            sTi = st_pool.tile([P, N], f32, tag="stp", name=f"sTi{c}")
            nc.scalar.activation(sTi[:], src_bc_i[:], Act.Prelu,
                                 bias=dstT[:, 2 * c + 1:2 * c + 2],
                                 alpha=pa_sb[:, 1:2])
            sT[h] = (sTr, sTi)
            sqr = sq_pool.tile([P, N], f32, tag="sqp", name=f"sqr{c}")
            nc.gpsimd.tensor_tensor(sqr[:], sTr[:], sTr[:], op=Alu.mult)
            sqi = sq_pool.tile([P, N], f32, tag="sqp", name=f"sqi{c}")
            nc.gpsimd.tensor_tensor(sqi[:], sTi[:], sTi[:], op=Alu.mult)
            nc.gpsimd.tensor_tensor(m2p[:, h, :], sqr[:], sqi[:], op=Alu.add)

        nc.scalar.activation(lnm2p[:].rearrange("p a b -> p (a b)"),
                             m2p[:].rearrange("p a b -> p (a b)"),
                             Act.Ln, bias=eps_col[:, 0:1])
        nc.scalar.activation(magp[:].rearrange("p a b -> p (a b)"),
                             lnm2p[:].rearrange("p a b -> p (a b)"),
                             Act.Exp, scale=0.5)
        for h in range(2):
            c = 2 * pair + h
            e_scr = e_pool.tile([P, N], bf16, tag="epool", name=f"e{c}")
            nc.scalar.activation(e_scr[:], magp[:, h, :], Act.Exp,
                                 accum_out=cs_cols[:, c:c + 1])
        nc.vector.scalar_tensor_tensor(dp[:].rearrange("p a b -> p (a b)"),
                                       lnm2p[:].rearrange("p a b -> p (a b)"),
                                       -0.5,
                                       magp[:].rearrange("p a b -> p (a b)"),
                                       op0=Alu.mult, op1=Alu.add)
        nc.scalar.activation(erp[:].rearrange("p a b -> p (a b)"),
                             dp[:].rearrange("p a b -> p (a b)"), Act.Exp)
        for h in range(2):
            c = 2 * pair + h
            sTr, sTi = sT[h]
            nc.vector.reciprocal(inv_cols[:, c:c + 1], cs_cols[:, c:c + 1])
            nc.vector.scalar_tensor_tensor(ar_t[c][:], sTr[:],
                                           inv_cols[:, c:c + 1], erp[:, h, :],
                                           op0=Alu.mult, op1=Alu.mult)
            nc.vector.scalar_tensor_tensor(ai_t[c][:], sTi[:],
                                           inv_cols[:, c:c + 1], erp[:, h, :],
                                           op0=Alu.mult, op1=Alu.mult)
            nc.vector.tensor_scalar(mai_t[c][:], ai_t[c][:], -1.0, None,
                                    op0=Alu.mult)

    # ---------------- era 2/3: PE diag passes + extraction + tail -------
    cp_pool = ctx.enter_context(tc.tile_pool(name="cpp", bufs=2))
    z_pool = ctx.enter_context(tc.tile_pool(name="zp", bufs=4))
    msl_pool = ctx.enter_context(tc.tile_pool(name="mslp", bufs=2))
    o_pool = ctx.enter_context(tc.tile_pool(name="op", bufs=4))

    with tc.tile_pool(name="diag_ps", bufs=2, space="PSUM") as dps:
        for k in range(K1):
            ps = dps.tile([P, CH, 2, P], f32, tag="dps", name=f"ps{k}")
            for c in range(CH):
                lt = l_tiles[(k, c)]
                last = (c == CH - 1)
                for ib in range(CH):
                    ibs = slice(ib * P, (ib + 1) * P)
                    # mm1: ar x (Lr|Li) -> (t0 += ar*Lr, t1 += ar*Li)
                    nc.tensor.matmul(ps[:, ib, :, :], ar_t[c][:, ibs],
                                     lt[:, :, ibs],
                                     start=(c == 0 and ib % 2 == 0),
                                     stop=False, skip_group_check=True)
                    # mm2: (-ai) x Li -> t0  (=> t0 = sum ar*Lr - ai*Li)
                    nc.tensor.matmul(ps[:, ib, 0, :], mai_t[c][:, ibs],
                                     lt[:, 1, ibs], start=False, stop=last,
                                     skip_group_check=True)
                    # mm3: ai x Lr -> t1     (=> t1 = sum ar*Li + ai*Lr)
                    nc.tensor.matmul(ps[:, ib, 1, :], ai_t[c][:, ibs],
                                     lt[:, 0, ibs], start=False, stop=last,
                                     skip_group_check=True)
            if k < K1 - 1:
                # extraction: ACT psum->bf16, DVE mask-mult (2x), DVE reduce
                cpb = cp_pool.tile([P, CH * 2, P], bf16, tag="cpp",
                                   name=f"cp{k}")
                nc.scalar.activation(cpb[:].rearrange("p a b -> p (a b)"),
                                     ps[:].rearrange("p a b c -> p (a b c)"),
                                     Act.Copy)
                mskd = cp_pool.tile([P, CH * 2, P], bf16, tag="cpp",
                                    name=f"mk{k}")
                mask_bc = mask_bf[:].unsqueeze(1).broadcast_to([P, CH * 2, P])
                nc.vector.tensor_tensor(mskd[:], cpb[:], mask_bc, op=Alu.mult)
                nc.vector.reduce_sum(
                    slAll[:, :, :, k].rearrange("p a b -> p (a b)"),
                    mskd[:], axis=X)
            else:
                # last pass: extract per ib and run the tail immediately so
                # it overlaps the remaining extraction instead of serializing
                cpb = cp_pool.tile([P, CH * 2, P], bf16, tag="cpp",
                                   name=f"cp{k}")
                nc.scalar.activation(cpb[:].rearrange("p a b -> p (a b)"),
                                     ps[:].rearrange("p a b c -> p (a b c)"),
                                     Act.Copy)
                mask_bc2 = mask_bf[:].unsqueeze(1).broadcast_to([P, 2, P])
                for ib in range(CH):
                    mskd = cp_pool.tile([P, 2, P], bf16, tag="cps",
                                        name=f"mk{k}_{ib}")
                    nc.vector.tensor_tensor(mskd[:], cpb[:, 2 * ib:2 * ib + 2,
                                                         :],
                                            mask_bc2, op=Alu.mult)
                    nc.vector.reduce_sum(slAll[:, ib, :, k], mskd[:], axis=X)
                    _tail_ib(nc, Alu, X, ib, slAll, msl_pool, z_pool, o_pool,
                             As_t, Bs_t, out_r, out_i, P, K1, OC, bf16, f32)


def _tail_ib(nc, Alu, X, ib, slAll, msl_pool, z_pool, o_pool,
             As_t, Bs_t, out_r, out_i, P, K1, OC, bf16, f32):
    slr = slAll[:, ib, 0, :]                      # [128, 5]
    sli = slAll[:, ib, 1, :]
    msli = msl_pool.tile([P, K1], f32, tag="mslp", name=f"msli{ib}")
    nc.vector.tensor_scalar(msli[:], sli, -1.0, None, op0=Alu.mult)
    slr_bc = slr.unsqueeze(2).broadcast_to([P, K1, OC])
    sli_bc = sli.unsqueeze(2).broadcast_to([P, K1, OC])
    msli_bc = msli[:].unsqueeze(2).broadcast_to([P, K1, OC])

    zr = z_pool.tile([P, 2, K1, OC], bf16, tag="zp", name=f"zr{ib}")
    zi = z_pool.tile([P, 2, K1, OC], bf16, tag="zp", name=f"zi{ib}")
    nc.gpsimd.tensor_tensor(zr[:, 0, :, :], As_t[ib][:], slr_bc, op=Alu.mult)
    nc.gpsimd.tensor_tensor(zr[:, 1, :, :], Bs_t[ib][:], msli_bc, op=Alu.mult)
    nc.gpsimd.tensor_tensor(zi[:, 0, :, :], As_t[ib][:], sli_bc, op=Alu.mult)
    nc.gpsimd.tensor_tensor(zi[:, 1, :, :], Bs_t[ib][:], slr_bc, op=Alu.mult)
    orc = o_pool.tile([P, OC], f32, tag="op", name=f"or{ib}")
    oic = o_pool.tile([P, OC], f32, tag="op", name=f"oi{ib}")
    nc.vector.reduce_sum(
        orc[:], zr[:].rearrange("p s k o -> p o (s k)"), axis=X)
    nc.vector.reduce_sum(
        oic[:], zi[:].rearrange("p s k o -> p o (s k)"), axis=X)
    nc.sync.dma_start(out_r[ib * P:(ib + 1) * P, :], orc[:])
    nc.sync.dma_start(out_i[ib * P:(ib + 1) * P, :], oic[:])


def _host_prep(inputs):
    import ml_dtypes
    BF = ml_dtypes.bfloat16
    f = lambda k: np.ascontiguousarray(np.asarray(inputs[k], dtype=np.float32))
    X_real, X_imag = f("X_real"), f("X_imag")
    w_real, w_imag = f("w_real"), f("w_imag")
    aw_real, aw_imag = f("aw_real"), f("aw_imag")
    ab_real = float(np.asarray(inputs["ab_real"]))
    ab_imag = float(np.asarray(inputs["ab_imag"]))
    pa_real = float(np.asarray(inputs["pa_real"]))
    pa_imag = float(np.asarray(inputs["pa_imag"]))

    # lcat[b]: [k][c][j 128][t 2][i 1024] with t0 = Lr^T, t1 = Li^T
    Lr = np.asarray(inputs["L_real"], dtype=np.float32)
    Li = np.asarray(inputs["L_imag"], dtype=np.float32)
    LrT = Lr.transpose(0, 1, 3, 2).reshape(B, K1, CH, P, N)
    LiT = Li.transpose(0, 1, 3, 2).reshape(B, K1, CH, P, N)
    lcat = np.ascontiguousarray(
        np.stack([LrT, LiT], axis=4).astype(BF))     # [B, K1, CH, P, 2, N]

    ws_r, wd_r = aw_real[:C], aw_real[C:]
    ws_i, wd_i = aw_imag[:C], aw_imag[C:]
    w4 = np.stack([
        np.concatenate([ws_r, -ws_i]),
        np.concatenate([ws_i, ws_r]),
        np.concatenate([wd_r, -wd_i]),
        np.concatenate([wd_i, wd_r]),
    ], axis=1).astype(np.float32)

    wr_t = w_real.transpose(1, 0, 2).reshape(C, K1 * OC)
    wi_t = w_imag.transpose(1, 0, 2).reshape(C, K1 * OC)
    wcat = np.concatenate([wr_t, wi_t], axis=0).astype(BF)

    pa_cols = np.stack([np.full(P, pa_real), np.full(P, pa_imag)],
                       axis=1).astype(np.float32)
    ab2 = np.array([[ab_real, ab_imag]], dtype=np.float32)

    in_maps = []
    for b in range(NCORES):
        in_maps.append({
            "x_real": X_real[b], "x_imag": X_imag[b],
            "lcat": lcat[b],
            "w4": w4, "wcat": wcat,
            "pa_cols": pa_cols, "ab2": ab2,
        })
    return in_maps


def kernel(**inputs):
    import os
    from concourse import bass_utils

    if "nc" not in _CACHE:
        _CACHE["nc"] = _build_nc()
    nc = _CACHE["nc"]
    in_maps = _host_prep(inputs)
    trace = os.environ.get("KERNEL_TRACE", "0") == "1"
    res = bass_utils.run_bass_kernel_spmd(nc, in_maps,
                                          core_ids=list(range(NCORES)),
                                          trace=trace)
    _CACHE["last_result"] = res
    out_r = np.stack([res.results[b]["out_r"] for b in range(NCORES)])
    out_i = np.stack([res.results[b]["out_i"] for b in range(NCORES)])
    return out_r, out_i
